# revision 48
# baseline (speedup 1.0000x reference)
"""Bahdanau attention with coverage — Trainium2 Bass/Tile kernel.

Shapes (hardcoded): B=32, T=4096, DE=256, DD=128, fp32 in/out.
Sharding: data-parallel over batch across 8 NeuronCores (4 examples/core);
tiny weights (W_a, U_a, v_a) replicated; no collectives.

Token layout: example tokens t = p*32 + i map to SBUF [partition p, col i],
so every enc/dec DMA moves 32KB-contiguous HBM runs per partition.

Per-core pipeline (single streaming pass over enc/dec, per example):
  - enc/dec loaded once, directly as float32r tiles (bit-identical; the PE
    rounds fp32r inputs internally) via chunk-granular ~0.5MB sub-DMAs
  - per 512-token chunk:
      PE-transpose (fp32r, via identity) enc/dec -> encT0/encT1/decT [128,512]
      hT = W_a0.T@encT0 + W_a1.T@encT1 + U_a.T@decT     (fp32r, N=512)
      PSUM->SBUF copies: encT0 on DVE, encT1/decT on ACT (engine balance)
      ACT tanh PSUM->SBUF; e_col[128t,1] = tanhT_s.T @ v_a  (4x N=1 fp32)
      ACT exp -> expe_r (f32r; no max-subtraction needed: |e| <= ~5)
      ctx_row[1,256] += expe_col.T @ enc_r   (fp32r N=256, 1-col weight load,
                                              single PSUM bank, one group)
  - epilogue: S broadcast to all partitions via ones[128,128]@rowsum matmul,
    invS = 1/S, attn = expe*invS, ctx = ctx_row*invS, coverage = attn + cov.

PE matmuls carry at most 1 semaphore wait in walrus codegen; building with
bacc.Bacc() + nc.compile() runs the wait-splitting legalization passes.
PSUM banks: transposes 3, h 2, e/S 2, ctx 1 (start=True clears has_written
flags bank-wide, so the long-lived ctx accumulator gets a bank to itself).
"""

import sys

if "/opt/trn_rl_repo" not in sys.path:
    sys.path.insert(0, "/opt/trn_rl_repo")

import numpy as np

import concourse.bacc as bacc
import concourse.bass as bass
import concourse.mybir as mybir
import concourse.tile as tile
from concourse.bass_utils import run_bass_kernel_spmd

F32 = mybir.dt.float32
F32R = mybir.dt.float32r
AF = mybir.ActivationFunctionType

B, T, DE, DD = 32, 4096, 256, 128
NCORES = 8
BPC = B // NCORES          # examples per core
CHUNK = 512                # tokens per chunk
NSUB = CHUNK // 128        # 128-token subtiles per chunk
NCH = T // CHUNK           # chunks per example
NTILES = T // 128          # exp_e columns per example

# fp32r gives full-rate PE matmuls (1 cyc/row at N>=256) at ~tf32 precision
# on the h/score path; the context path stays exact fp32.
H_DT = F32R


def build_program() -> bass.Bass:
    nc = bacc.Bacc()

    enc = nc.dram_tensor("enc", [BPC, T, DE], F32, kind="ExternalInput")
    dec = nc.dram_tensor("dec", [BPC, T, DD], F32, kind="ExternalInput")
    cov = nc.dram_tensor("cov", [BPC, T], F32, kind="ExternalInput")
    wa = nc.dram_tensor("wa", [DE, DD], F32, kind="ExternalInput")
    ua = nc.dram_tensor("ua", [DD, DD], F32, kind="ExternalInput")
    va = nc.dram_tensor("va", [DD, 1], F32, kind="ExternalInput")
    identw = nc.dram_tensor("identw", [128, 128], F32, kind="ExternalInput")

    ctx_out = nc.dram_tensor("ctx", [BPC, DE], F32, kind="ExternalOutput")
    attn_out = nc.dram_tensor("attn", [BPC, T], F32, kind="ExternalOutput")
    covo_out = nc.dram_tensor("covo", [BPC, T], F32, kind="ExternalOutput")

    with tile.TileContext(nc) as tc:
        with (
            tc.tile_pool(name="singles", bufs=1) as singles,
            tc.tile_pool(name="enc_pool", bufs=2) as enc_pool,
            tc.tile_pool(name="dec_pool", bufs=2) as dec_pool,
            tc.tile_pool(name="tT_pool", bufs=4) as tT_pool,
            tc.tile_pool(name="tanh_pool", bufs=4) as tanh_pool,
            tc.tile_pool(name="ex_pool", bufs=3) as ex_pool,
            tc.tile_pool(name="small", bufs=3) as small,
            tc.tile_pool(name="tp_psum", bufs=3, space="PSUM") as tp_psum,
            tc.tile_pool(name="h_psum", bufs=2, space="PSUM") as h_psum,
            tc.tile_pool(name="e_psum", bufs=2, space="PSUM") as e_psum,
            tc.tile_pool(name="ctx_psum", bufs=1, space="PSUM") as ctx_psum,
        ):
            # DMA issue order matters (HWDGE is FIFO): identity first (gates
            # the first transposes), then example-0 chunk-0/1 data, then the
            # small constants, then the rest of example 0.
            ident_r = singles.tile([128, 128], H_DT, name="ident_r")
            nc.scalar.dma_start(out=ident_r, in_=identw[:].bitcast(H_DT))

            pre_enc = enc_pool.tile([128, NTILES, DE], H_DT, name="enc_r", tag="enc_r")
            pre_dec = dec_pool.tile([128, NTILES, DD], H_DT, name="dec_r", tag="dec_r")
            enc0_src = enc[0].bitcast(H_DT).rearrange("(p i) e -> p i e", p=128)
            dec0_src = dec[0].bitcast(H_DT).rearrange("(p i) e -> p i e", p=128)
            for i0, ilen in ((0, 1), (1, NSUB - 1), (NSUB, NSUB)):
                nc.sync.dma_start(
                    out=pre_enc[:, i0 : i0 + ilen, :],
                    in_=enc0_src[:, i0 : i0 + ilen, :],
                )
                nc.sync.dma_start(
                    out=pre_dec[:, i0 : i0 + ilen, :],
                    in_=dec0_src[:, i0 : i0 + ilen, :],
                )

            # ---- constants / weights (once per core) ----
            wa_sb = singles.tile([128, 2, DD], F32, name="wa_sb")
            nc.scalar.dma_start(out=wa_sb, in_=wa[:].rearrange("(c p) d -> p c d", p=128))
            ua_sb = singles.tile([128, DD], F32, name="ua_sb")
            nc.scalar.dma_start(out=ua_sb, in_=ua[:])
            va_sb = singles.tile([128, 1], F32, name="va_sb")
            nc.scalar.dma_start(out=va_sb, in_=va[:])
            ones_sb = singles.tile([128, 128], F32, name="ones_sb")
            nc.vector.memset(ones_sb, 1.0)

            # rest of example 0
            for h in range((NTILES - 2 * NSUB) // NSUB):
                i0 = 2 * NSUB + h * NSUB
                nc.sync.dma_start(
                    out=pre_enc[:, i0 : i0 + NSUB, :],
                    in_=enc0_src[:, i0 : i0 + NSUB, :],
                )
                nc.sync.dma_start(
                    out=pre_dec[:, i0 : i0 + NSUB, :],
                    in_=dec0_src[:, i0 : i0 + NSUB, :],
                )
            # fp32r operands must be produced by rounding instructions
            # (BIR verifier) — make one-time rounded weight copies.
            wa_r = singles.tile([128, 2, DD], H_DT, name="wa_r")
            nc.vector.tensor_copy(wa_r, wa_sb)
            ua_r = singles.tile([128, DD], H_DT, name="ua_r")
            nc.vector.tensor_copy(ua_r, ua_sb)

            for b in range(BPC):
                expe_r = ex_pool.tile([128, NTILES], H_DT, name="expe_r")
                ctxp = ctx_psum.tile([1, DE], F32, name="ctxp")
                cov_sb = small.tile([128, NTILES], F32, name="cov_sb")
                nc.scalar.dma_start(
                    out=cov_sb, in_=cov[b].rearrange("(p i) -> p i", p=128)
                )
                # example tiles filled by ~1MB sub-DMAs (2 chunks each);
                # example 0 was issued before the constant preamble.
                if b == 0:
                    enc_r, dec_r = pre_enc, pre_dec
                else:
                    enc_r = enc_pool.tile(
                        [128, NTILES, DE], H_DT, name="enc_r", tag="enc_r"
                    )
                    dec_r = dec_pool.tile(
                        [128, NTILES, DD], H_DT, name="dec_r", tag="dec_r"
                    )
                    enc_src = enc[b].bitcast(H_DT).rearrange("(p i) e -> p i e", p=128)
                    dec_src = dec[b].bitcast(H_DT).rearrange("(p i) e -> p i e", p=128)
                    for h in range(NTILES // NSUB):
                        i0 = h * NSUB
                        nc.sync.dma_start(
                            out=enc_r[:, i0 : i0 + NSUB, :],
                            in_=enc_src[:, i0 : i0 + NSUB, :],
                        )
                        nc.sync.dma_start(
                            out=dec_r[:, i0 : i0 + NSUB, :],
                            in_=dec_src[:, i0 : i0 + NSUB, :],
                        )

                for c in range(NCH):
                    sub0 = c * NSUB

                    # --- transposes: [token, feat] -> [feat, token] ---
                    encT0 = tT_pool.tile([128, CHUNK], H_DT, name="encT0")
                    encT1 = tT_pool.tile([128, CHUNK], H_DT, name="encT1")
                    decT = tT_pool.tile([128, CHUNK], H_DT, name="decT")
                    for dst, src_lo, src_hi, is_enc in (
                        (encT0, 0, 128, True),
                        (encT1, 128, 256, True),
                        (decT, 0, 128, False),
                    ):
                        tp = tp_psum.tile([128, CHUNK], H_DT, name="tp", tag="tp")
                        src = enc_r if is_enc else dec_r
                        for s in range(NSUB):
                            nc.tensor.transpose(
                                tp[:, s * 128 : (s + 1) * 128],
                                src[:, sub0 + s, src_lo:src_hi],
                                ident_r,
                            )
                        if src_lo == 0 and is_enc:
                            nc.vector.tensor_copy(dst, tp)
                        else:
                            nc.scalar.activation(out=dst, in_=tp, func=AF.Copy)

                    # --- hT = W_a0.T@encT0 + W_a1.T@encT1 + U_a.T@decT ---
                    hp = h_psum.tile([128, CHUNK], F32, name="hp")
                    nc.tensor.matmul(
                        hp, wa_r[:, 0, :], encT0, start=True, stop=False
                    )
                    nc.tensor.matmul(
                        hp, wa_r[:, 1, :], encT1, start=False, stop=False
                    )
                    nc.tensor.matmul(
                        hp, ua_r, decT, start=False, stop=True
                    )

                    tanhT = tanh_pool.tile([128, CHUNK], F32, name="tanhT")
                    nc.scalar.activation(out=tanhT, in_=hp, func=AF.Tanh)

                    # --- scores e per 128-token subtile -> token-partition cols ---
                    ep = e_psum.tile([128, NSUB], F32, name="ep")
                    for s in range(NSUB):
                        nc.tensor.matmul(
                            ep[:, s : s + 1],
                            tanhT[:, s * 128 : (s + 1) * 128],
                            va_sb,
                            start=True,
                            stop=True,
                        )
                    nc.scalar.activation(
                        out=expe_r[:, c * NSUB : (c + 1) * NSUB], in_=ep, func=AF.Exp
                    )

                    # --- unnormalized context accumulation (row form):
                    # ctx_row[1, DE] += expe_col.T @ enc_r  (1-col weight load)
                    for s in range(NSUB):
                        nc.tensor.matmul(
                            ctxp,
                            expe_r[:, c * NSUB + s : c * NSUB + s + 1],
                            enc_r[:, sub0 + s, :],
                            start=(c == 0 and s == 0),
                            stop=(c == NCH - 1 and s == NSUB - 1),
                            skip_group_check=True,
                        )

                # ---- epilogue: softmax normalization + outputs ----
                rowsum = small.tile([128, 1], F32, name="rowsum")
                nc.vector.tensor_reduce(
                    rowsum,
                    expe_r.bitcast(F32),
                    axis=mybir.AxisListType.X,
                    op=mybir.AluOpType.add,
                )
                sp = e_psum.tile([128, NSUB], F32, name="ep", tag="ep")[:, 0:1]
                nc.tensor.matmul(sp, ones_sb, rowsum, start=True, stop=True)
                invs = small.tile([128, 1], F32, name="invs")
                nc.vector.reciprocal(invs, sp)

                attn_sb = small.tile([128, NTILES], F32, name="attn_sb")
                nc.vector.tensor_scalar_mul(attn_sb, expe_r.bitcast(F32), invs)
                ctx_sb = small.tile([1, DE], F32, name="ctx_sb")
                nc.vector.tensor_scalar_mul(ctx_sb, ctxp, invs[0:1, :])
                covo_sb = small.tile([128, NTILES], F32, name="covo_sb")
                nc.vector.tensor_add(covo_sb, attn_sb, cov_sb)

                # outputs: ctx is ready first; attn/covo go out on the two
                # HWDGE rings (SP + ACT) in parallel
                nc.scalar.dma_start(out=ctx_out[b][None, :], in_=ctx_sb)
                nc.sync.dma_start(
                    out=attn_out[b].rearrange("(p i) -> p i", p=128), in_=attn_sb
                )
                nc.scalar.dma_start(
                    out=covo_out[b].rearrange("(p i) -> p i", p=128), in_=covo_sb
                )

    nc.compile()
    return nc


_NC_CACHE = None


def _get_program():
    global _NC_CACHE
    if _NC_CACHE is None:
        _NC_CACHE = build_program()
    return _NC_CACHE


def run(inputs: dict, trace: bool = False):
    """Run on 8 cores. Returns ((ctx, attn, cov), exec_time_ns|None)."""
    enc = np.ascontiguousarray(np.asarray(inputs["encoder_outputs"], dtype=np.float32))
    dec = np.ascontiguousarray(np.asarray(inputs["decoder_outputs"], dtype=np.float32))
    cov = np.ascontiguousarray(
        np.asarray(inputs["coverage_vector"], dtype=np.float32).reshape(B, T)
    )
    wa = np.ascontiguousarray(np.asarray(inputs["W_a"], dtype=np.float32))
    ua = np.ascontiguousarray(np.asarray(inputs["U_a"], dtype=np.float32))
    va = np.ascontiguousarray(np.asarray(inputs["v_a"], dtype=np.float32))

    ident_np = np.eye(128, dtype=np.float32)
    nc = _get_program()
    core_ids = list(range(NCORES))
    in_maps = []
    for i in core_ids:
        sl = slice(i * BPC, (i + 1) * BPC)
        in_maps.append(
            {
                "enc": enc[sl],
                "dec": dec[sl],
                "cov": cov[sl],
                "wa": wa,
                "ua": ua,
                "va": va,
                "identw": ident_np,
            }
        )

    try:
        res = run_bass_kernel_spmd(nc, in_maps, core_ids, trace=trace)
    except Exception:
        # transient device states (e.g. NRT exec-unit unrecoverable) usually
        # clear on retry
        import time as _time

        _time.sleep(10)
        res = run_bass_kernel_spmd(nc, in_maps, core_ids, trace=trace)
    outs = res.results

    ctx = np.concatenate([outs[i]["ctx"] for i in core_ids], axis=0)
    attn = np.concatenate([outs[i]["attn"] for i in core_ids], axis=0)
    covo = np.concatenate([outs[i]["covo"] for i in core_ids], axis=0)
    return (
        (
            ctx.astype(np.float32),
            attn.reshape(B, T, 1).astype(np.float32),
            covo.reshape(B, T, 1).astype(np.float32),
        ),
        res.exec_time_ns,
    )


def kernel(**inputs):
    (ctx, attn, covo), _ = run(inputs, trace=False)
    return (ctx, attn, covo)


# revision 49
# speedup vs baseline: 1.0021x; 1.0021x over previous
"""Bahdanau attention with coverage — Trainium2 Bass/Tile kernel.

Shapes (hardcoded): B=32, T=4096, DE=256, DD=128, fp32 in/out.
Sharding: data-parallel over batch across 8 NeuronCores (4 examples/core);
tiny weights (W_a, U_a, v_a) replicated; no collectives.

Token layout: example tokens t = p*32 + i map to SBUF [partition p, col i],
so every enc/dec DMA moves 32KB-contiguous HBM runs per partition.

Per-core pipeline (single streaming pass over enc/dec, per example):
  - enc/dec loaded once, directly as float32r tiles (bit-identical; the PE
    rounds fp32r inputs internally) via chunk-granular ~0.5MB sub-DMAs
  - per 512-token chunk:
      PE-transpose (fp32r, via identity) enc/dec -> encT0/encT1/decT [128,512]
      hT = W_a0.T@encT0 + W_a1.T@encT1 + U_a.T@decT     (fp32r, N=512)
      PSUM->SBUF copies: encT0 on DVE, encT1/decT on ACT (engine balance)
      ACT tanh PSUM->SBUF; e_col[128t,1] = tanhT_s.T @ v_a  (4x N=1 fp32)
      ACT exp -> expe_r (f32r; no max-subtraction needed: |e| <= ~5)
      ctx_row[1,256] += expe_col.T @ enc_r   (fp32r N=256, 1-col weight load,
                                              single PSUM bank, one group)
  - epilogue: S broadcast to all partitions via ones[128,128]@rowsum matmul,
    invS = 1/S, attn = expe*invS, ctx = ctx_row*invS, coverage = attn + cov.

PE matmuls carry at most 1 semaphore wait in walrus codegen; building with
bacc.Bacc() + nc.compile() runs the wait-splitting legalization passes.
PSUM banks: transposes 3, h 2, e/S 2, ctx 1 (start=True clears has_written
flags bank-wide, so the long-lived ctx accumulator gets a bank to itself).
"""

import sys

if "/opt/trn_rl_repo" not in sys.path:
    sys.path.insert(0, "/opt/trn_rl_repo")

import numpy as np

import concourse.bacc as bacc
import concourse.bass as bass
import concourse.mybir as mybir
import concourse.tile as tile
from concourse.bass_utils import run_bass_kernel_spmd

F32 = mybir.dt.float32
F32R = mybir.dt.float32r
AF = mybir.ActivationFunctionType

B, T, DE, DD = 32, 4096, 256, 128
NCORES = 8
BPC = B // NCORES          # examples per core
CHUNK = 512                # tokens per chunk
NSUB = CHUNK // 128        # 128-token subtiles per chunk
NCH = T // CHUNK           # chunks per example
NTILES = T // 128          # exp_e columns per example

# fp32r gives full-rate PE matmuls (1 cyc/row at N>=256) at ~tf32 precision
# on the h/score path; the context path stays exact fp32.
H_DT = F32R


def build_program() -> bass.Bass:
    nc = bacc.Bacc()

    enc = nc.dram_tensor("enc", [BPC, T, DE], F32, kind="ExternalInput")
    dec = nc.dram_tensor("dec", [BPC, T, DD], F32, kind="ExternalInput")
    cov = nc.dram_tensor("cov", [BPC, T], F32, kind="ExternalInput")
    wa = nc.dram_tensor("wa", [DE, DD], F32, kind="ExternalInput")
    ua = nc.dram_tensor("ua", [DD, DD], F32, kind="ExternalInput")
    va = nc.dram_tensor("va", [DD, 1], F32, kind="ExternalInput")
    identw = nc.dram_tensor("identw", [128, 128], F32, kind="ExternalInput")

    ctx_out = nc.dram_tensor("ctx", [BPC, DE], F32, kind="ExternalOutput")
    attn_out = nc.dram_tensor("attn", [BPC, T], F32, kind="ExternalOutput")
    covo_out = nc.dram_tensor("covo", [BPC, T], F32, kind="ExternalOutput")

    with tile.TileContext(nc) as tc:
        with (
            tc.tile_pool(name="singles", bufs=1) as singles,
            tc.tile_pool(name="enc_pool", bufs=2) as enc_pool,
            tc.tile_pool(name="dec_pool", bufs=2) as dec_pool,
            tc.tile_pool(name="tT_pool", bufs=4) as tT_pool,
            tc.tile_pool(name="tanh_pool", bufs=4) as tanh_pool,
            tc.tile_pool(name="ex_pool", bufs=3) as ex_pool,
            tc.tile_pool(name="small", bufs=3) as small,
            tc.tile_pool(name="tp_psum", bufs=3, space="PSUM") as tp_psum,
            tc.tile_pool(name="h_psum", bufs=2, space="PSUM") as h_psum,
            tc.tile_pool(name="e_psum", bufs=2, space="PSUM") as e_psum,
            tc.tile_pool(name="ctx_psum", bufs=1, space="PSUM") as ctx_psum,
        ):
            # DMA issue order matters (HWDGE is FIFO): identity first (gates
            # the first transposes), then example-0 chunk-0/1 data, then the
            # small constants, then the rest of example 0.
            ident_r = singles.tile([128, 128], H_DT, name="ident_r")
            nc.scalar.dma_start(out=ident_r, in_=identw[:].bitcast(H_DT))

            pre_enc = enc_pool.tile([128, NTILES, DE], H_DT, name="enc_r", tag="enc_r")
            pre_dec = dec_pool.tile([128, NTILES, DD], H_DT, name="dec_r", tag="dec_r")
            enc0_src = enc[0].bitcast(H_DT).rearrange("(p i) e -> p i e", p=128)
            dec0_src = dec[0].bitcast(H_DT).rearrange("(p i) e -> p i e", p=128)
            for i0, ilen in ((0, 1), (1, NSUB - 1), (NSUB, NSUB)):
                nc.sync.dma_start(
                    out=pre_enc[:, i0 : i0 + ilen, :],
                    in_=enc0_src[:, i0 : i0 + ilen, :],
                )
                nc.sync.dma_start(
                    out=pre_dec[:, i0 : i0 + ilen, :],
                    in_=dec0_src[:, i0 : i0 + ilen, :],
                )

            # ---- constants / weights (once per core) ----
            wa_sb = singles.tile([128, 2, DD], F32, name="wa_sb")
            nc.scalar.dma_start(out=wa_sb, in_=wa[:].rearrange("(c p) d -> p c d", p=128))
            ua_sb = singles.tile([128, DD], F32, name="ua_sb")
            nc.scalar.dma_start(out=ua_sb, in_=ua[:])
            va_sb = singles.tile([128, 1], F32, name="va_sb")
            nc.scalar.dma_start(out=va_sb, in_=va[:])
            ones_sb = singles.tile([128, 128], F32, name="ones_sb")
            nc.vector.memset(ones_sb, 1.0)

            # rest of example 0
            for h in range((NTILES - 2 * NSUB) // NSUB):
                i0 = 2 * NSUB + h * NSUB
                nc.sync.dma_start(
                    out=pre_enc[:, i0 : i0 + NSUB, :],
                    in_=enc0_src[:, i0 : i0 + NSUB, :],
                )
                nc.sync.dma_start(
                    out=pre_dec[:, i0 : i0 + NSUB, :],
                    in_=dec0_src[:, i0 : i0 + NSUB, :],
                )
            # fp32r operands must be produced by rounding instructions
            # (BIR verifier) — make one-time rounded weight copies.
            wa_r = singles.tile([128, 2, DD], H_DT, name="wa_r")
            nc.vector.tensor_copy(wa_r, wa_sb)
            ua_r = singles.tile([128, DD], H_DT, name="ua_r")
            nc.vector.tensor_copy(ua_r, ua_sb)

            for b in range(BPC):
                expe_r = ex_pool.tile([128, NTILES], H_DT, name="expe_r")
                ctxp = ctx_psum.tile([1, DE], F32, name="ctxp")
                cov_sb = small.tile([128, NTILES], F32, name="cov_sb")
                nc.scalar.dma_start(
                    out=cov_sb, in_=cov[b].rearrange("(p i) -> p i", p=128)
                )
                # example tiles filled by ~1MB sub-DMAs (2 chunks each);
                # example 0 was issued before the constant preamble.
                if b == 0:
                    enc_r, dec_r = pre_enc, pre_dec
                else:
                    enc_r = enc_pool.tile(
                        [128, NTILES, DE], H_DT, name="enc_r", tag="enc_r"
                    )
                    dec_r = dec_pool.tile(
                        [128, NTILES, DD], H_DT, name="dec_r", tag="dec_r"
                    )
                    enc_src = enc[b].bitcast(H_DT).rearrange("(p i) e -> p i e", p=128)
                    dec_src = dec[b].bitcast(H_DT).rearrange("(p i) e -> p i e", p=128)
                    for h in range(NTILES // NSUB):
                        i0 = h * NSUB
                        nc.sync.dma_start(
                            out=enc_r[:, i0 : i0 + NSUB, :],
                            in_=enc_src[:, i0 : i0 + NSUB, :],
                        )
                        nc.sync.dma_start(
                            out=dec_r[:, i0 : i0 + NSUB, :],
                            in_=dec_src[:, i0 : i0 + NSUB, :],
                        )

                for c in range(NCH):
                    sub0 = c * NSUB

                    # --- transposes: [token, feat] -> [feat, token] ---
                    encT0 = tT_pool.tile([128, CHUNK], H_DT, name="encT0")
                    encT1 = tT_pool.tile([128, CHUNK], H_DT, name="encT1")
                    decT = tT_pool.tile([128, CHUNK], H_DT, name="decT")
                    for dst, src_lo, src_hi, is_enc in (
                        (encT0, 0, 128, True),
                        (encT1, 128, 256, True),
                        (decT, 0, 128, False),
                    ):
                        tp = tp_psum.tile([128, CHUNK], H_DT, name="tp", tag="tp")
                        src = enc_r if is_enc else dec_r
                        for s in range(NSUB):
                            nc.tensor.transpose(
                                tp[:, s * 128 : (s + 1) * 128],
                                src[:, sub0 + s, src_lo:src_hi],
                                ident_r,
                            )
                        if src_lo == 0 and is_enc:
                            nc.vector.tensor_copy(dst, tp)
                        else:
                            nc.scalar.activation(out=dst, in_=tp, func=AF.Copy)

                    # --- hT = W_a0.T@encT0 + W_a1.T@encT1 + U_a.T@decT ---
                    hp = h_psum.tile([128, CHUNK], F32, name="hp")
                    nc.tensor.matmul(
                        hp, wa_r[:, 0, :], encT0, start=True, stop=False
                    )
                    nc.tensor.matmul(
                        hp, wa_r[:, 1, :], encT1, start=False, stop=False
                    )
                    nc.tensor.matmul(
                        hp, ua_r, decT, start=False, stop=True
                    )

                    tanhT = tanh_pool.tile([128, CHUNK], F32, name="tanhT")
                    nc.scalar.activation(out=tanhT, in_=hp, func=AF.Tanh)

                    # --- scores e per 128-token subtile -> token-partition cols ---
                    ep = e_psum.tile([128, NSUB], F32, name="ep")
                    for s in range(NSUB):
                        nc.tensor.matmul(
                            ep[:, s : s + 1],
                            tanhT[:, s * 128 : (s + 1) * 128],
                            va_sb,
                            start=True,
                            stop=True,
                        )
                    nc.scalar.activation(
                        out=expe_r[:, c * NSUB : (c + 1) * NSUB], in_=ep, func=AF.Exp
                    )

                    # --- unnormalized context accumulation (row form):
                    # ctx_row[1, DE] += expe_col.T @ enc_r  (1-col weight load)
                    for s in range(NSUB):
                        nc.tensor.matmul(
                            ctxp,
                            expe_r[:, c * NSUB + s : c * NSUB + s + 1],
                            enc_r[:, sub0 + s, :],
                            start=(c == 0 and s == 0),
                            stop=(c == NCH - 1 and s == NSUB - 1),
                            skip_group_check=True,
                        )

                # ---- epilogue: softmax normalization + outputs ----
                rowsum = small.tile([128, 1], F32, name="rowsum")
                nc.vector.tensor_reduce(
                    rowsum,
                    expe_r.bitcast(F32),
                    axis=mybir.AxisListType.X,
                    op=mybir.AluOpType.add,
                )
                sp = e_psum.tile([128, NSUB], F32, name="ep", tag="ep")[:, 0:1]
                nc.tensor.matmul(sp, ones_sb, rowsum, start=True, stop=True)
                invs = small.tile([128, 1], F32, name="invs")
                nc.vector.reciprocal(invs, sp)

                # normalization fans out across three engines
                attn_sb = small.tile([128, NTILES], F32, name="attn_sb")
                nc.scalar.activation(
                    out=attn_sb, in_=expe_r.bitcast(F32), func=AF.Copy, scale=invs
                )
                ctx_sb = small.tile([1, DE], F32, name="ctx_sb")
                nc.vector.tensor_scalar_mul(ctx_sb, ctxp, invs[0:1, :])
                covo_sb = small.tile([128, NTILES], F32, name="covo_sb")
                nc.gpsimd.tensor_add(covo_sb, attn_sb, cov_sb)

                # outputs: ctx is ready first; attn/covo go out on the two
                # HWDGE rings (SP + ACT) in parallel
                nc.scalar.dma_start(out=ctx_out[b][None, :], in_=ctx_sb)
                nc.sync.dma_start(
                    out=attn_out[b].rearrange("(p i) -> p i", p=128), in_=attn_sb
                )
                nc.scalar.dma_start(
                    out=covo_out[b].rearrange("(p i) -> p i", p=128), in_=covo_sb
                )

    nc.compile()
    return nc


_NC_CACHE = None


def _get_program():
    global _NC_CACHE
    if _NC_CACHE is None:
        _NC_CACHE = build_program()
    return _NC_CACHE


def run(inputs: dict, trace: bool = False):
    """Run on 8 cores. Returns ((ctx, attn, cov), exec_time_ns|None)."""
    enc = np.ascontiguousarray(np.asarray(inputs["encoder_outputs"], dtype=np.float32))
    dec = np.ascontiguousarray(np.asarray(inputs["decoder_outputs"], dtype=np.float32))
    cov = np.ascontiguousarray(
        np.asarray(inputs["coverage_vector"], dtype=np.float32).reshape(B, T)
    )
    wa = np.ascontiguousarray(np.asarray(inputs["W_a"], dtype=np.float32))
    ua = np.ascontiguousarray(np.asarray(inputs["U_a"], dtype=np.float32))
    va = np.ascontiguousarray(np.asarray(inputs["v_a"], dtype=np.float32))

    ident_np = np.eye(128, dtype=np.float32)
    nc = _get_program()
    core_ids = list(range(NCORES))
    in_maps = []
    for i in core_ids:
        sl = slice(i * BPC, (i + 1) * BPC)
        in_maps.append(
            {
                "enc": enc[sl],
                "dec": dec[sl],
                "cov": cov[sl],
                "wa": wa,
                "ua": ua,
                "va": va,
                "identw": ident_np,
            }
        )

    try:
        res = run_bass_kernel_spmd(nc, in_maps, core_ids, trace=trace)
    except Exception:
        # transient device states (e.g. NRT exec-unit unrecoverable) usually
        # clear on retry
        import time as _time

        _time.sleep(10)
        res = run_bass_kernel_spmd(nc, in_maps, core_ids, trace=trace)
    outs = res.results

    ctx = np.concatenate([outs[i]["ctx"] for i in core_ids], axis=0)
    attn = np.concatenate([outs[i]["attn"] for i in core_ids], axis=0)
    covo = np.concatenate([outs[i]["covo"] for i in core_ids], axis=0)
    return (
        (
            ctx.astype(np.float32),
            attn.reshape(B, T, 1).astype(np.float32),
            covo.reshape(B, T, 1).astype(np.float32),
        ),
        res.exec_time_ns,
    )


def kernel(**inputs):
    (ctx, attn, covo), _ = run(inputs, trace=False)
    return (ctx, attn, covo)


# revision 50
# speedup vs baseline: 1.0381x; 1.0360x over previous
"""Bahdanau attention with coverage — Trainium2 Bass/Tile kernel.

Shapes (hardcoded): B=32, T=4096, DE=256, DD=128, fp32 in/out.
Sharding: data-parallel over batch across 8 NeuronCores (4 examples/core);
tiny weights (W_a, U_a, v_a) replicated; no collectives.

Token layout: example tokens t = p*32 + i map to SBUF [partition p, col i],
so every enc/dec DMA moves 32KB-contiguous HBM runs per partition.

Per-core pipeline (single streaming pass over enc/dec, per example):
  - enc/dec loaded once, directly as float32r tiles (bit-identical; the PE
    rounds fp32r inputs internally) via chunk-granular ~0.5MB sub-DMAs
  - per 512-token chunk:
      PE-transpose (fp32r, via identity) enc/dec -> encT0/encT1/decT [128,512]
      hT = W_a0.T@encT0 + W_a1.T@encT1 + U_a.T@decT     (fp32r, N=512)
      PSUM->SBUF copies: encT0 on DVE, encT1/decT on ACT (engine balance)
      ACT tanh PSUM->SBUF; e_col[128t,1] = tanhT_s.T @ v_a  (4x N=1 fp32)
      ACT exp -> expe_r (f32r; no max-subtraction needed: |e| <= ~5)
      ctx_row[1,256] += expe_col.T @ enc_r   (fp32r N=256, 1-col weight load,
                                              single PSUM bank, one group)
  - epilogue: S broadcast to all partitions via ones[128,128]@rowsum matmul,
    invS = 1/S, attn = expe*invS, ctx = ctx_row*invS, coverage = attn + cov.

PE matmuls carry at most 1 semaphore wait in walrus codegen; building with
bacc.Bacc() + nc.compile() runs the wait-splitting legalization passes.
PSUM banks: transposes 3, h 2, e/S 2, ctx 1 (start=True clears has_written
flags bank-wide, so the long-lived ctx accumulator gets a bank to itself).
"""

import sys

if "/opt/trn_rl_repo" not in sys.path:
    sys.path.insert(0, "/opt/trn_rl_repo")

import numpy as np

import concourse.bacc as bacc
import concourse.bass as bass
import concourse.mybir as mybir
import concourse.tile as tile
from concourse.bass_utils import run_bass_kernel_spmd

F32 = mybir.dt.float32
F32R = mybir.dt.float32r
AF = mybir.ActivationFunctionType

B, T, DE, DD = 32, 4096, 256, 128
NCORES = 8
BPC = B // NCORES          # examples per core
CHUNK = 512                # tokens per chunk
NSUB = CHUNK // 128        # 128-token subtiles per chunk
NCH = T // CHUNK           # chunks per example
NTILES = T // 128          # exp_e columns per example

# fp32r gives full-rate PE matmuls (1 cyc/row at N>=256) at ~tf32 precision
# on the h/score path; the context path stays exact fp32.
H_DT = F32R


def build_program() -> bass.Bass:
    nc = bacc.Bacc()

    enc = nc.dram_tensor("enc", [BPC, T, DE], F32, kind="ExternalInput")
    dec = nc.dram_tensor("dec", [BPC, T, DD], F32, kind="ExternalInput")
    cov = nc.dram_tensor("cov", [BPC, T], F32, kind="ExternalInput")
    wa = nc.dram_tensor("wa", [DE, DD], F32, kind="ExternalInput")
    ua = nc.dram_tensor("ua", [DD, DD], F32, kind="ExternalInput")
    va = nc.dram_tensor("va", [DD, 1], F32, kind="ExternalInput")
    identw = nc.dram_tensor("identw", [128, 128], F32, kind="ExternalInput")

    ctx_out = nc.dram_tensor("ctx", [BPC, DE], F32, kind="ExternalOutput")
    attn_out = nc.dram_tensor("attn", [BPC, T], F32, kind="ExternalOutput")
    covo_out = nc.dram_tensor("covo", [BPC, T], F32, kind="ExternalOutput")

    with tile.TileContext(nc) as tc:
        with (
            tc.tile_pool(name="singles", bufs=1) as singles,
            tc.tile_pool(name="enc_pool", bufs=2) as enc_pool,
            tc.tile_pool(name="dec_pool", bufs=2) as dec_pool,
            tc.tile_pool(name="tT_pool", bufs=4) as tT_pool,
            tc.tile_pool(name="tanh_pool", bufs=4) as tanh_pool,
            tc.tile_pool(name="ex_pool", bufs=3) as ex_pool,
            tc.tile_pool(name="small", bufs=3) as small,
            tc.tile_pool(name="tp_psum", bufs=3, space="PSUM") as tp_psum,
            tc.tile_pool(name="h_psum", bufs=2, space="PSUM") as h_psum,
            tc.tile_pool(name="e_psum", bufs=2, space="PSUM") as e_psum,
            tc.tile_pool(name="ctx_psum", bufs=1, space="PSUM") as ctx_psum,
        ):
            # DMA issue order matters (HWDGE is FIFO): identity first (gates
            # the first transposes), then example-0 chunk-0/1 data, then the
            # small constants, then the rest of example 0.
            ident_r = singles.tile([128, 128], H_DT, name="ident_r")
            nc.scalar.dma_start(out=ident_r, in_=identw[:].bitcast(H_DT))

            pre_enc = enc_pool.tile([128, NTILES, DE], H_DT, name="enc_r", tag="enc_r")
            pre_dec = dec_pool.tile([128, NTILES, DD], H_DT, name="dec_r", tag="dec_r")
            enc0_src = enc[0].bitcast(H_DT).rearrange("(p i) e -> p i e", p=128)
            dec0_src = dec[0].bitcast(H_DT).rearrange("(p i) e -> p i e", p=128)
            for i0, ilen in ((0, 1), (1, NSUB - 1), (NSUB, NSUB)):
                nc.sync.dma_start(
                    out=pre_enc[:, i0 : i0 + ilen, :],
                    in_=enc0_src[:, i0 : i0 + ilen, :],
                )
                nc.sync.dma_start(
                    out=pre_dec[:, i0 : i0 + ilen, :],
                    in_=dec0_src[:, i0 : i0 + ilen, :],
                )

            # ---- constants / weights (once per core) ----
            wa_sb = singles.tile([128, 2, DD], F32, name="wa_sb")
            nc.scalar.dma_start(out=wa_sb, in_=wa[:].rearrange("(c p) d -> p c d", p=128))
            ua_sb = singles.tile([128, DD], F32, name="ua_sb")
            nc.scalar.dma_start(out=ua_sb, in_=ua[:])
            va_sb = singles.tile([128, 1], F32, name="va_sb")
            nc.scalar.dma_start(out=va_sb, in_=va[:])
            ones_sb = singles.tile([128, 128], F32, name="ones_sb")
            nc.vector.memset(ones_sb, 1.0)

            # rest of example 0
            for h in range((NTILES - 2 * NSUB) // NSUB):
                i0 = 2 * NSUB + h * NSUB
                nc.sync.dma_start(
                    out=pre_enc[:, i0 : i0 + NSUB, :],
                    in_=enc0_src[:, i0 : i0 + NSUB, :],
                )
                nc.sync.dma_start(
                    out=pre_dec[:, i0 : i0 + NSUB, :],
                    in_=dec0_src[:, i0 : i0 + NSUB, :],
                )
            # fp32r operands must be produced by rounding instructions
            # (BIR verifier) — make one-time rounded weight copies.
            wa_r = singles.tile([128, 2, DD], H_DT, name="wa_r")
            nc.vector.tensor_copy(wa_r, wa_sb)
            ua_r = singles.tile([128, DD], H_DT, name="ua_r")
            nc.vector.tensor_copy(ua_r, ua_sb)

            pending_epilogue = None
            for b in range(BPC):
                expe_r = ex_pool.tile([128, NTILES], H_DT, name="expe_r")
                ctxp = ctx_psum.tile([1, DE], F32, name="ctxp")
                cov_sb = small.tile([128, NTILES], F32, name="cov_sb")
                nc.scalar.dma_start(
                    out=cov_sb, in_=cov[b].rearrange("(p i) -> p i", p=128)
                )
                # example tiles filled by ~1MB sub-DMAs (2 chunks each);
                # example 0 was issued before the constant preamble.
                if b == 0:
                    enc_r, dec_r = pre_enc, pre_dec
                else:
                    enc_r = enc_pool.tile(
                        [128, NTILES, DE], H_DT, name="enc_r", tag="enc_r"
                    )
                    dec_r = dec_pool.tile(
                        [128, NTILES, DD], H_DT, name="dec_r", tag="dec_r"
                    )
                    enc_src = enc[b].bitcast(H_DT).rearrange("(p i) e -> p i e", p=128)
                    dec_src = dec[b].bitcast(H_DT).rearrange("(p i) e -> p i e", p=128)
                    for h in range(NTILES // NSUB):
                        i0 = h * NSUB
                        nc.sync.dma_start(
                            out=enc_r[:, i0 : i0 + NSUB, :],
                            in_=enc_src[:, i0 : i0 + NSUB, :],
                        )
                        nc.sync.dma_start(
                            out=dec_r[:, i0 : i0 + NSUB, :],
                            in_=dec_src[:, i0 : i0 + NSUB, :],
                        )

                for c in range(NCH):
                    sub0 = c * NSUB

                    # --- transposes: [token, feat] -> [feat, token] ---
                    encT0 = tT_pool.tile([128, CHUNK], H_DT, name="encT0")
                    encT1 = tT_pool.tile([128, CHUNK], H_DT, name="encT1")
                    decT = tT_pool.tile([128, CHUNK], H_DT, name="decT")
                    for dst, src_lo, src_hi, is_enc in (
                        (encT0, 0, 128, True),
                        (encT1, 128, 256, True),
                        (decT, 0, 128, False),
                    ):
                        tp = tp_psum.tile([128, CHUNK], H_DT, name="tp", tag="tp")
                        src = enc_r if is_enc else dec_r
                        for s in range(NSUB):
                            nc.tensor.transpose(
                                tp[:, s * 128 : (s + 1) * 128],
                                src[:, sub0 + s, src_lo:src_hi],
                                ident_r,
                            )
                        if src_lo == 0 and is_enc:
                            nc.vector.tensor_copy(dst, tp)
                        else:
                            nc.scalar.activation(out=dst, in_=tp, func=AF.Copy)
                        if pending_epilogue is not None:
                            pending_epilogue()
                            pending_epilogue = None

                    # --- hT = W_a0.T@encT0 + W_a1.T@encT1 + U_a.T@decT ---
                    hp = h_psum.tile([128, CHUNK], F32, name="hp")
                    nc.tensor.matmul(
                        hp, wa_r[:, 0, :], encT0, start=True, stop=False
                    )
                    nc.tensor.matmul(
                        hp, wa_r[:, 1, :], encT1, start=False, stop=False
                    )
                    nc.tensor.matmul(
                        hp, ua_r, decT, start=False, stop=True
                    )

                    tanhT = tanh_pool.tile([128, CHUNK], F32, name="tanhT")
                    nc.scalar.activation(out=tanhT, in_=hp, func=AF.Tanh)

                    # --- scores e per 128-token subtile -> token-partition cols ---
                    ep = e_psum.tile([128, NSUB], F32, name="ep")
                    for s in range(NSUB):
                        nc.tensor.matmul(
                            ep[:, s : s + 1],
                            tanhT[:, s * 128 : (s + 1) * 128],
                            va_sb,
                            start=True,
                            stop=True,
                        )
                    nc.scalar.activation(
                        out=expe_r[:, c * NSUB : (c + 1) * NSUB], in_=ep, func=AF.Exp
                    )

                    # --- unnormalized context accumulation (row form):
                    # ctx_row[1, DE] += expe_col.T @ enc_r  (1-col weight load)
                    for s in range(NSUB):
                        nc.tensor.matmul(
                            ctxp,
                            expe_r[:, c * NSUB + s : c * NSUB + s + 1],
                            enc_r[:, sub0 + s, :],
                            start=(c == 0 and s == 0),
                            stop=(c == NCH - 1 and s == NSUB - 1),
                            skip_group_check=True,
                        )

                # ---- epilogue: deferred; emitted after the NEXT example's
                # first transpose group so PE isn't stalled in program order
                # on the softmax barrier (S waits rowsum waits last exp)
                def make_epilogue(b=b, expe_r=expe_r, ctxp=ctxp, cov_sb=cov_sb):
                    def _epi():
                        rowsum = small.tile([128, 1], F32, name="rowsum")
                        nc.vector.tensor_reduce(
                            rowsum,
                            expe_r.bitcast(F32),
                            axis=mybir.AxisListType.X,
                            op=mybir.AluOpType.add,
                        )
                        sp = e_psum.tile([128, NSUB], F32, name="ep", tag="ep")[:, 0:1]
                        nc.tensor.matmul(sp, ones_sb, rowsum, start=True, stop=True)
                        invs = small.tile([128, 1], F32, name="invs")
                        nc.vector.reciprocal(invs, sp)

                        # normalization fans out across three engines
                        attn_sb = small.tile([128, NTILES], F32, name="attn_sb")
                        nc.scalar.activation(
                            out=attn_sb,
                            in_=expe_r.bitcast(F32),
                            func=AF.Copy,
                            scale=invs,
                        )
                        ctx_sb = small.tile([1, DE], F32, name="ctx_sb")
                        nc.vector.tensor_scalar_mul(ctx_sb, ctxp, invs[0:1, :])
                        covo_sb = small.tile([128, NTILES], F32, name="covo_sb")
                        nc.gpsimd.tensor_add(covo_sb, attn_sb, cov_sb)

                        nc.scalar.dma_start(out=ctx_out[b][None, :], in_=ctx_sb)
                        nc.sync.dma_start(
                            out=attn_out[b].rearrange("(p i) -> p i", p=128),
                            in_=attn_sb,
                        )
                        nc.scalar.dma_start(
                            out=covo_out[b].rearrange("(p i) -> p i", p=128),
                            in_=covo_sb,
                        )

                    return _epi

                pending_epilogue = make_epilogue()
            pending_epilogue()

    nc.compile()
    return nc


_NC_CACHE = None


def _get_program():
    global _NC_CACHE
    if _NC_CACHE is None:
        _NC_CACHE = build_program()
    return _NC_CACHE


def run(inputs: dict, trace: bool = False):
    """Run on 8 cores. Returns ((ctx, attn, cov), exec_time_ns|None)."""
    enc = np.ascontiguousarray(np.asarray(inputs["encoder_outputs"], dtype=np.float32))
    dec = np.ascontiguousarray(np.asarray(inputs["decoder_outputs"], dtype=np.float32))
    cov = np.ascontiguousarray(
        np.asarray(inputs["coverage_vector"], dtype=np.float32).reshape(B, T)
    )
    wa = np.ascontiguousarray(np.asarray(inputs["W_a"], dtype=np.float32))
    ua = np.ascontiguousarray(np.asarray(inputs["U_a"], dtype=np.float32))
    va = np.ascontiguousarray(np.asarray(inputs["v_a"], dtype=np.float32))

    ident_np = np.eye(128, dtype=np.float32)
    nc = _get_program()
    core_ids = list(range(NCORES))
    in_maps = []
    for i in core_ids:
        sl = slice(i * BPC, (i + 1) * BPC)
        in_maps.append(
            {
                "enc": enc[sl],
                "dec": dec[sl],
                "cov": cov[sl],
                "wa": wa,
                "ua": ua,
                "va": va,
                "identw": ident_np,
            }
        )

    try:
        res = run_bass_kernel_spmd(nc, in_maps, core_ids, trace=trace)
    except Exception:
        # transient device states (e.g. NRT exec-unit unrecoverable) usually
        # clear on retry
        import time as _time

        _time.sleep(10)
        res = run_bass_kernel_spmd(nc, in_maps, core_ids, trace=trace)
    outs = res.results

    ctx = np.concatenate([outs[i]["ctx"] for i in core_ids], axis=0)
    attn = np.concatenate([outs[i]["attn"] for i in core_ids], axis=0)
    covo = np.concatenate([outs[i]["covo"] for i in core_ids], axis=0)
    return (
        (
            ctx.astype(np.float32),
            attn.reshape(B, T, 1).astype(np.float32),
            covo.reshape(B, T, 1).astype(np.float32),
        ),
        res.exec_time_ns,
    )


def kernel(**inputs):
    (ctx, attn, covo), _ = run(inputs, trace=False)
    return (ctx, attn, covo)
